# revision 46
# baseline (speedup 1.0000x reference)
"""BlockGRUCell fused Trainium2 kernel.

Sharding: data-parallel over batch across 8 NeuronCores (weights replicated,
host-cast to bf16).  Dataflow is fully transposed ([features, batch] on-chip):
every matmul uses the weight matrix in natural [in, out] layout as the
stationary lhsT and the transposed activations [in, batch] as the moving rhs,
producing the next layer's transposed activations directly.  LayerNorm
reductions run over the partition axis via ones-vector matmuls; per-batch
stats are broadcast back across partitions with a rank-1 matmul.

v3: all matmul operands bf16 (half HBM traffic + fast weight load); stage-1
weights fully resident; dyn_W double-buffered per-group on the scalar HWDGE
queue; gru_W per-group tiles prefetched during stage 2; stage-2 LN interleaved
with stage-3 segments; LN stats on DVE only (no ACT table thrash); bf16
elementwise tail; batched bf16 output DMAs.
"""
import numpy as np
from contextlib import ExitStack

import concourse.bass as bass
import concourse.tile as tile
from concourse import bacc, mybir
from concourse._compat import with_exitstack
from concourse.bass_utils import run_bass_kernel_spmd

B, D, S, H, G = 2048, 4096, 1024, 1024, 8
A = 128
DG = D // G            # 512
ING = DG + 3 * H       # 3584
NCORES = 8
BC = B // NCORES       # 256 batch rows per core
EPS = 1e-5

F32 = mybir.dt.float32
F32R = mybir.dt.float32r
BF16 = mybir.dt.bfloat16
AF = mybir.ActivationFunctionType
OP = mybir.AluOpType

# vecs column layout (all per-partition tiled: col j holds v[j*128 + p])
NB_H = H // 128        # 8
NB_D = D // 128        # 32
NB_Z = 3 * D // 128    # 96
C_BD, C_GD, C_BED = 0, NB_H, 2 * NB_H
C_BS, C_GS, C_BES = 3 * NB_H, 4 * NB_H, 5 * NB_H
C_BA, C_GA, C_BEA = 6 * NB_H, 7 * NB_H, 8 * NB_H
C_DYNB, C_GDYN, C_BEDYN = 9 * NB_H, 9 * NB_H + NB_D, 9 * NB_H + 2 * NB_D
C_GRUB = 9 * NB_H + 3 * NB_D
NV = C_GRUB + NB_Z     # 264

# stage-2 LN group order == stage-3 first-use order.  Segment 0 only needs
# groups {0,2,5}; LN of the rest overlaps segment-0 compute.  ACT table
# switches (Silu <-> Sigmoid) cost 1.3us each, so the interleave is coarse:
# exactly two Silu batches and one Sigmoid run.
LN_FIRST = [0, 2, 5]
LN_REST = [3, 6, 1, 4, 7]
GRU_EARLY = [0, 2, 5, 3, 6, 1]   # prefetched during stage 2
# gru 4 reuses gru 0's tile (g0 last read in seg 2, needed seg 4); gru 7
# reuses gru 3's tile (last read seg 3, needed seg 5).
GRU_SLOT = {0: "g0", 2: "g2", 5: "g5", 3: "g3", 6: "g6", 1: "g1",
            4: "g0", 7: "g3"}


@with_exitstack
def _emit(ctx: ExitStack, tc: tile.TileContext, ins: dict, outT: bass.AP):
    nc = tc.nc
    ctx.enter_context(nc.allow_low_precision(
        reason="bf16 weights/activations with fp32 PSUM accumulation"))

    persist = ctx.enter_context(tc.tile_pool(name="persist", bufs=1))
    sqp = ctx.enter_context(tc.tile_pool(name="sqp", bufs=2))
    small = ctx.enter_context(tc.tile_pool(name="small", bufs=1))
    consts = ctx.enter_context(tc.tile_pool(name="consts", bufs=1))
    mm_ps = ctx.enter_context(tc.tile_pool(name="mm_ps", bufs=1, space="PSUM"))
    st_ps = tc.alloc_tile_pool(name="st_ps", bufs=1, space="PSUM")
    bc_ps = tc.alloc_tile_pool(name="bc_ps", bufs=1, space="PSUM")
    dynpool = tc.alloc_tile_pool(name="dynpool", bufs=1)
    s1w = tc.alloc_tile_pool(name="s1w", bufs=1)
    s1pool = tc.alloc_tile_pool(name="s1pool", bufs=1)
    s1upool = tc.alloc_tile_pool(name="s1upool", bufs=2)

    # ---- tiny constants (vecs rides the scalar queue: it feeds DVE/ACT,
    # not the first matmuls, and keeps the sync queue free for weights) ----
    vecs_sb = persist.tile([128, NV], F32, name="vecs")
    nc.scalar.dma_start(out=vecs_sb, in_=ins["vecs"])
    ones_col_f = consts.tile([128, 1], F32)
    nc.vector.memset(ones_col_f, 1.0)
    ones_col = consts.tile([128, 1], BF16)
    nc.vector.tensor_copy(ones_col, ones_col_f)
    ones_row_f = consts.tile([1, 128], F32)
    nc.vector.memset(ones_row_f, 1.0)
    ones_row = consts.tile([1, 128], F32R)
    nc.vector.tensor_copy(ones_row, ones_row_f)
    eps_t = consts.tile([1, 1], F32)
    nc.vector.memset(eps_t, EPS)

    # ---- stage-1 inputs + weights (sync queue), in compute order ----
    def _wtile(name, kt):
        return s1w.tile([128, kt, H], BF16, name=name)

    actionT_sb = s1pool.tile([128, 1, BC], BF16, name="actionT")
    nc.sync.dma_start(out=actionT_sb,
                      in_=ins["actionT"].rearrange("(t p) b -> p t b", p=128))
    wa_t = _wtile("wa", 1)
    nc.sync.dma_start(out=wa_t,
                      in_=ins["W_a"].rearrange("(kk p) n -> p kk n", p=128))
    stochT_sb = s1pool.tile([128, S // 128, BC], BF16, name="stochT")
    nc.sync.dma_start(out=stochT_sb,
                      in_=ins["stochT"].rearrange("(t p) b -> p t b", p=128))
    ws_t = _wtile("ws", S // 128)
    for q in range(2):
        nc.sync.dma_start(
            out=ws_t[:, q * 4:(q + 1) * 4, :],
            in_=ins["W_s"][q * 512:(q + 1) * 512, :]
            .rearrange("(kk p) n -> p kk n", p=128))
    deterT_sb = persist.tile([128, NB_D, BC], BF16, name="deterT")
    _dT = ins["deterT"].rearrange("(t p) b -> p t b", p=128)
    for q in range(4):
        nc.sync.dma_start(out=deterT_sb[:, q * 8:(q + 1) * 8, :],
                          in_=_dT[:, q * 8:(q + 1) * 8, :])
    wd_t = _wtile("wd", NB_D)
    for q in range(8):
        nc.sync.dma_start(
            out=wd_t[:, q * 4:(q + 1) * 4, :],
            in_=ins["W_d"][q * 512:(q + 1) * 512, :]
            .rearrange("(kk p) n -> p kk n", p=128))

    # dyn_W streams as 16 half-group tiles through the otherwise-idle gpsimd
    # SWDGE queue (no HOL interference with the sync weight stream or ACT
    # compute), triple-buffered so transfers stay two halves ahead.
    NDH = 3
    dynh = [dynpool.tile([128, 14, DG], BF16, name=f"dynh{i}")
            for i in range(NDH)]

    def dyn_dma(h, eng=None):
        g, hf = divmod(h, 2)
        (eng or nc.gpsimd).dma_start(
            out=dynh[h % NDH],
            in_=ins["dyn_W"][g, hf * 1792:(hf + 1) * 1792, :]
            .rearrange("(kk p) n -> p kk n", p=128))

    # first three halves go on the sync queue AFTER the stage-1 weights so
    # they don't steal DMA bandwidth from them at startup; later halves use
    # the gpsimd SWDGE queue where their WAR waits can't block anything.
    for h in range(NDH):
        dyn_dma(h, eng=nc.sync)

    x_sb = persist.tile([128, 24, BC], BF16, name="x")
    y_sb = persist.tile([128, NB_D, BC], BF16, name="y_sb")

    mm_tags = [f"up{i}" for i in range(4)]

    def mm_tile(i):
        return mm_ps.tile([128, 2 * BC], F32, name=mm_tags[i % 4])[:, :BC]

    z_state = {}

    def z_pair(i):
        tags = [(mm_ps, t) for t in mm_tags]
        if "z2" in z_state:
            tags += [(z_state["z2"], f"z2_{j}") for j in range(4)]
        pool, t = tags[i % len(tags)]
        return pool.tile([128, 2 * BC], F32, name=t)

    def stats_finish(ssum, ssq, nfeat):
        """LN stats entirely on DVE (no ACT table thrash), broadcast via PE.
        Returns bf16 [128, 2*BC] mean/rstd tiles (stat duplicated per half)."""
        mr2 = small.tile([1, 4 * BC], F32R, name="mr2")
        mean, mean_b = mr2[:, :BC], mr2[:, BC:2 * BC]
        rstd, rstd_b = mr2[:, 2 * BC:3 * BC], mr2[:, 3 * BC:]
        nc.vector.tensor_scalar_mul(mean, ssum, 1.0 / nfeat)
        nc.vector.tensor_copy(mean_b, mean)
        m2 = small.tile([1, BC], F32, name="m2")
        nc.vector.tensor_mul(m2, mean, mean)
        var = small.tile([1, BC], F32, name="var")
        nc.vector.tensor_scalar(var, ssq, 1.0 / nfeat, None, OP.mult, OP.bypass)
        nc.vector.tensor_sub(var, var, m2)
        std = small.tile([1, BC], F32, name="std")
        nc.scalar.activation(std, var, AF.Sqrt, bias=eps_t, scale=1.0)
        nc.vector.reciprocal(rstd, std)
        nc.vector.tensor_copy(rstd_b, rstd)
        bc0 = bc_ps.tile([128, 2 * BC], F32, name="bc0")
        nc.tensor.matmul(bc0, ones_row, mr2[:, :2 * BC], start=True, stop=True)
        bc1 = bc_ps.tile([128, 2 * BC], F32, name="bc1")
        nc.tensor.matmul(bc1, ones_row, mr2[:, 2 * BC:], start=True, stop=True)
        mr0 = sqp.tile([128, 2 * BC], BF16, name="mr0")
        nc.vector.tensor_copy(mr0, bc0)
        mr1 = sqp.tile([128, 2 * BC], BF16, name="mr1")
        nc.vector.tensor_copy(mr1, bc1)
        return mr0, mr1

    def ln_apply(u_sb_pair, meanB2, rstdB2, g_cols, be_cols, dst_fn, pair_list,
                 sigmoid_set=False):
        """Paired LN apply: dst = silu(((u - m) * r) * gamma + beta).
        Sub/mul run on bf16 [128, 512] pairs (DVE 2x); silu on ACT per half.
        sigmoid_set=True computes silu as a*sigmoid(a) so the ACT table can
        stay on the sigmoid set while stage-3 gates run (no table thrash)."""
        for i in pair_list:
            tp = sqp.tile([128, 2 * BC], BF16, name="tp")
            nc.vector.tensor_sub(tp, u_sb_pair(i), meanB2)
            nc.vector.tensor_mul(tp, tp, rstdB2)
            for h in (0, 1):
                idx = i + h
                gcol = vecs_sb[:, g_cols + idx:g_cols + idx + 1]
                becol = vecs_sb[:, be_cols + idx:be_cols + idx + 1]
                th = tp[:, h * BC:(h + 1) * BC]
                if not sigmoid_set:
                    nc.scalar.activation(dst_fn(idx), th, AF.Silu,
                                         bias=becol, scale=gcol)
                else:
                    sg = sqp.tile([128, BC], BF16, name="lsg")
                    nc.scalar.activation(sg, th, AF.Sigmoid,
                                         bias=becol, scale=gcol)
                    af = sqp.tile([128, BC], BF16, name="laf")
                    nc.vector.tensor_scalar(af, th, gcol, becol,
                                            OP.mult, OP.add)
                    nc.vector.tensor_mul(dst_fn(idx), af, sg)

    # ================= stage 1: three input projections =================
    stage1 = [
        (wa_t, actionT_sb, 1, C_BA, C_GA, C_BEA, 16),
        (ws_t, stochT_sb, S // 128, C_BS, C_GS, C_BES, 8),
        (wd_t, deterT_sb, NB_D, C_BD, C_GD, C_BED, 0),
    ]
    for w_t, rhs_sb, KT, bcol, gcol, becol, xoff in stage1:
        u_sb = s1upool.tile([128, NB_H, BC], BF16, name="u_sb")
        ssum = st_ps.tile([1, BC], F32, name="ssum")
        ssq = st_ps.tile([1, BC], F32, name="ssq")
        for c in range(2):  # H in two chunks of 4 m-tiles
            psums = [mm_tile(m) for m in range(4)]
            for k in range(KT):
                for m in range(4):
                    nc.tensor.matmul(
                        psums[m],
                        w_t[:, k, c * 512 + m * 128:c * 512 + (m + 1) * 128],
                        rhs_sb[:, k, :],
                        start=(k == 0), stop=(k == KT - 1),
                    )
            for m in range(4):
                mt = c * 4 + m
                ut = u_sb[:, mt, :]
                nc.vector.tensor_scalar_add(ut, psums[m],
                                            vecs_sb[:, bcol + mt:bcol + mt + 1])
                usq = sqp.tile([128, BC], BF16, name="usq")
                nc.scalar.square(usq, ut)
                nc.tensor.matmul(ssum, ones_col, ut,
                                 start=(mt == 0), stop=(mt == NB_H - 1))
                nc.tensor.matmul(ssq, ones_col, usq,
                                 start=(mt == 0), stop=(mt == NB_H - 1))
        meanB2, rstdB2 = stats_finish(ssum, ssq, H)
        ln_apply(lambda i, u=u_sb: u[:, i:i + 2, :].rearrange("p a b -> p (a b)"),
                 meanB2, rstdB2, gcol, becol,
                 lambda idx, xoff=xoff: x_sb[:, xoff + idx, :],
                 [0, 2, 4, 6])
    s1upool.release()
    s1pool.release()
    s1w.release()

    # gru prefetch lives where the stage-1 weights were (6 slots, two reused)
    grupool = tc.alloc_tile_pool(name="grupool", bufs=1)
    gru_t = {}

    def gru_dma(gf):
        gru_t[gf] = grupool.tile([128, 4, 3 * DG], BF16, name=GRU_SLOT[gf])
        nc.sync.dma_start(
            out=gru_t[gf],
            in_=ins["gru_W"][gf].rearrange("(kk p) n -> p kk n", p=128))

    # ================= stage 2: block-diagonal dyn layer =================
    yssum = st_ps.tile([1, BC], F32, name="ssum")
    yssq = st_ps.tile([1, BC], F32, name="ssq")
    KT2 = ING // 128  # 28

    for g in range(G):
        psums = [mm_tile(m) for m in range(4)]
        for k in range(KT2):
            # at the start of half hh, prefetch half hh+2: its buffer holds
            # half hh-1, whose reader matmuls are already emitted (WAR safe)
            if k % 14 == 0:
                hh = 2 * g + k // 14
                if hh >= 1 and hh + 2 < 16:
                    dyn_dma(hh + 2)
            wt = dynh[(2 * g + k // 14) % NDH]
            rhs = deterT_sb[:, g * 4 + k, :] if k < 4 else x_sb[:, k - 4, :]
            for m in range(4):
                nc.tensor.matmul(
                    psums[m],
                    wt[:, k % 14, m * 128:(m + 1) * 128],
                    rhs,
                    start=(k == 0), stop=(k == KT2 - 1),
                )
        for m in range(4):
            ft = g * 4 + m
            yt = y_sb[:, ft, :]
            nc.vector.tensor_scalar_add(yt, psums[m],
                                        vecs_sb[:, C_DYNB + ft:C_DYNB + ft + 1])
            ysq = sqp.tile([128, BC], BF16, name="usq")
            nc.scalar.square(ysq, yt)
            nc.tensor.matmul(yssum, ones_col, yt,
                             start=(ft == 0), stop=(ft == NB_D - 1))
            nc.tensor.matmul(yssq, ones_col, ysq,
                             start=(ft == 0), stop=(ft == NB_D - 1))
        if 1 <= g <= len(GRU_EARLY):
            gru_dma(GRU_EARLY[g - 1])

    meanB2, rstdB2 = stats_finish(yssum, yssq, D)
    bc_ps.release()
    st_ps.release()
    z2_ps = tc.alloc_tile_pool(name="z2_ps", bufs=1, space="PSUM")
    z_state["z2"] = z2_ps
    outpool = tc.alloc_tile_pool(name="outpool", bufs=2)
    s3tmp = tc.alloc_tile_pool(name="s3tmp", bufs=2)

    def ln_group(g, sigmoid_set=False):
        ln_apply(lambda i: y_sb[:, i:i + 2, :].rearrange("p a b -> p (a b)"),
                 meanB2, rstdB2, C_GDYN, C_BEDYN, lambda idx: y_sb[:, idx, :],
                 [g * 4, g * 4 + 2], sigmoid_set=sigmoid_set)

    # ================= stage 3: GRU gates + output =================
    # zflat f-tiles: reset tj, cand tj+32, update tj+64; block gf = f*128//1536.
    zcnt = 0

    def segment(seg):
        nonlocal zcnt
        tj0 = seg * 4
        tjs = list(range(tj0, tj0 + 4))
        panels = []
        for off in range(3):
            f0 = tj0 * 128 + off * 4096
            gf, col0 = f0 // 1536, f0 % 1536
            panels.append((gru_t[gf], gf, col0))

        def zmm(off, tj):
            nonlocal zcnt
            co = (tj - tj0) * 128
            wp, gf, col0 = panels[off]
            zp = z_pair(zcnt)[:, :BC]
            zcnt += 1
            for k in range(4):
                nc.tensor.matmul(zp, wp[:, k, col0 + co:col0 + co + 128],
                                 y_sb[:, gf * 4 + k, :],
                                 start=(k == 0), stop=(k == 3))
            return zp

        o_st = outpool.tile([128, 4, BC], BF16, name="o_st")
        r, uu, rc = {}, {}, {}
        for tj in tjs:  # reset gate: sigmoid batch
            zp = zmm(0, tj)
            r[tj] = s3tmp.tile([128, BC], BF16, name=f"r{tj % 2}")
            nc.scalar.activation(r[tj], zp, AF.Sigmoid,
                                 bias=vecs_sb[:, C_GRUB + tj:C_GRUB + tj + 1],
                                 scale=1.0)
        for tj in tjs:  # update gate: sigmoid batch
            zp = zmm(2, tj)
            uu[tj] = s3tmp.tile([128, BC], BF16, name=f"uu{tj % 2}")
            nc.scalar.activation(uu[tj], zp, AF.Sigmoid,
                                 bias=vecs_sb[:, C_GRUB + 64 + tj:C_GRUB + 64 + tj + 1],
                                 scale=1.0)
        for tj in tjs:  # cand pre-act: DVE + gpsimd only
            zp = zmm(1, tj)
            cp = s3tmp.tile([128, BC], BF16, name="cp")
            nc.vector.tensor_scalar_add(cp, zp,
                                        vecs_sb[:, C_GRUB + 32 + tj:C_GRUB + 32 + tj + 1])
            rc[tj] = s3tmp.tile([128, BC], BF16, name=f"rc{tj % 2}")
            nc.gpsimd.tensor_mul(rc[tj], r[tj], cp)
        for tj in tjs:  # tanh via 2*sigmoid(2x)-1; out = uu*(c-d)+d
            ss = s3tmp.tile([128, BC], BF16, name="cc")
            nc.scalar.activation(ss, rc[tj], AF.Sigmoid, bias=0.0, scale=2.0)
            dt_ = deterT_sb[:, tj, :]
            cd = s3tmp.tile([128, BC], BF16, name="cd")
            nc.vector.scalar_tensor_tensor(cd, ss, 2.0, dt_, OP.mult, OP.subtract)
            o = o_st[:, tj - tj0, :]
            nc.vector.scalar_tensor_tensor(o, cd, 1.0, uu[tj], OP.subtract, OP.mult)
            nc.vector.tensor_add(o, o, dt_)
        nc.sync.dma_start(
            out=outT.rearrange("(t p) b -> p t b", p=128)[:, tj0:tj0 + 4, :],
            in_=o_st)

    # coarse interleave: Silu batch for segment-0's groups, segment 0,
    # remaining Silu batch (overlaps seg-0 gates), then segments 1..7.
    for g in LN_FIRST:
        ln_group(g)
    segment(0)
    for g in LN_REST:
        ln_group(g, sigmoid_set=True)
    for s in range(1, 8):
        segment(s)
        if s == 2:
            gru_dma(4)  # reuses group-0's slot; its readers (segs 0-2) are emitted
        elif s == 3:
            gru_dma(7)  # reuses group-3's slot; its readers (segs 1-3) are emitted

    z2_ps.release()
    s3tmp.release()
    outpool.release()
    grupool.release()
    dynpool.release()


_CACHE = {}


def _build():
    if "nc" in _CACHE:
        return _CACHE["nc"]
    nc = bacc.Bacc("TRN2", target_bir_lowering=False, debug=False,
                   num_devices=NCORES)
    ins = {}
    for name, shape, dt in [
        ("deterT", [D, BC], BF16), ("stochT", [S, BC], BF16),
        ("actionT", [A, BC], BF16),
        ("W_d", [D, H], BF16), ("W_s", [S, H], BF16), ("W_a", [A, H], BF16),
        ("dyn_W", [G, ING, DG], BF16), ("gru_W", [G, DG, 3 * DG], BF16),
        ("vecs", [128, NV], F32),
    ]:
        ins[name] = nc.dram_tensor(name, shape, dt, kind="ExternalInput").ap()
    outT = nc.dram_tensor("outT", [D, BC], BF16, kind="ExternalOutput").ap()
    with tile.TileContext(nc) as tc:
        _emit(tc, ins, outT)
    nc.compile()
    _CACHE["nc"] = nc
    return nc


def _col_tile(v):
    """[L] -> [128, L//128] with col t holding v[t*128 + p]."""
    return np.ascontiguousarray(v.reshape(-1, 128).T.astype(np.float32))


def _make_vecs(b_d, g_d, be_d, b_s, g_s, be_s, b_a, g_a, be_a,
               dyn_b, g_dyn, be_dyn, gru_b):
    gru_adj = np.array(gru_b, dtype=np.float32).copy()
    gru_adj[2 * D:] -= 1.0
    cols = [b_d, g_d, be_d, b_s, g_s, be_s, b_a, g_a, be_a,
            dyn_b, g_dyn, be_dyn, gru_adj]
    return np.concatenate([_col_tile(np.asarray(c)) for c in cols], axis=1), gru_adj


def kernel(deter, stoch, action,
           W_d, b_d, g_d, be_d,
           W_s, b_s, g_s, be_s,
           W_a, b_a, g_a, be_a,
           dyn_W, dyn_b, g_dyn, be_dyn,
           gru_W, gru_b):
    nc = _build()

    import ml_dtypes
    bf16 = ml_dtypes.bfloat16
    deterT = np.asarray(deter, dtype=np.float32).T.astype(bf16)
    stochT = np.asarray(stoch, dtype=np.float32).T.astype(bf16)
    actionT = np.asarray(action, dtype=np.float32).T.astype(bf16)
    vecs, gru_adj = _make_vecs(b_d, g_d, be_d, b_s, g_s, be_s, b_a, g_a, be_a,
                               dyn_b, g_dyn, be_dyn, gru_b)
    shared = {
        "W_d": np.ascontiguousarray(np.asarray(W_d).astype(bf16)),
        "W_s": np.ascontiguousarray(np.asarray(W_s).astype(bf16)),
        "W_a": np.ascontiguousarray(np.asarray(W_a).astype(bf16)),
        "dyn_W": np.ascontiguousarray(np.asarray(dyn_W).astype(bf16)),
        "gru_W": np.ascontiguousarray(np.asarray(gru_W).astype(bf16)),
        "vecs": vecs,
    }
    in_maps = []
    for c in range(NCORES):
        sl = slice(c * BC, (c + 1) * BC)
        m = dict(shared)
        m["deterT"] = np.ascontiguousarray(deterT[:, sl])
        m["stochT"] = np.ascontiguousarray(stochT[:, sl])
        m["actionT"] = np.ascontiguousarray(actionT[:, sl])
        in_maps.append(m)

    import os
    kw = {}
    if os.environ.get("BASS_TMPDIR"):
        kw["tmpdir"] = os.environ["BASS_TMPDIR"]
    res = run_bass_kernel_spmd(nc, in_maps, list(range(NCORES)), **kw)
    global LAST_RES
    LAST_RES = res
    outT = np.concatenate(
        [res.results[c]["outT"].astype(np.float32) for c in range(NCORES)],
        axis=1)
    return np.ascontiguousarray(outT.T)


LAST_RES = None
